# revision 22
# baseline (speedup 1.0000x reference)
"""Trainium2 Bass kernel for nn_BHS_SAGE (GNN message passing + dueling head).

Node-chunk sharding: core c owns nodes [128c, 128(c+1)) of ALL 128 graphs
(instead of 16 whole graphs).  The SAGE stages (pool-MLP, edge max-agg,
self+neigh matmul) see the same per-core work either way, but the dueling
head only needs this core's 128-node slice of W_adv/W_v1 (2.4 MB instead of
the full 19.9 MB replicated), and its matmuls run at M=128 (full PE rows).

Each core returns its per-graph head partial sums [128 g, 76] in fp32; the
host sums the 8 partials and applies the dueling tail (76 -> 12 outputs per
graph, ~60 KFLOP total vs ~9 GFLOP on device).  An on-device ReduceScatter
was measured at ~50 us of firmware latency for 39 KB and abandoned.

Per-core pipeline, 8 slabs (each slab = 16 dst nodes x 32 graph-groups):
  A. z = W_pool_blockdiag @ xe-slab (4-graph packed, 1024-col PSUM blocks)
  B. agg = relu(max_d z + b); two drain flavours balanced across engines:
       direct slab:   DVE reduce_max from PSUM (1x), bias+relu after (4x TS)
       assisted slab: ACT relu+bias drain PSUM->SBUF bf16, then a
                      slab-batched TT-max tree (2x_1p) on DVE
  D. h = relu(W_self x + W_neigh agg + b) per graph quadrant (ACT drain)
  E. head psum[128 g, 76] += ht[:, j].T @ whead[:, j]  (16 matmuls/slab,
     accumulated across all 128 j; hidden under the slab drains)
"""

import numpy as np

B, N, F, H, DEG = 128, 1024, 32, 128, 16
NCORES = 8
NC_ = N // NCORES         # 128 nodes per core chunk
BL = B // NCORES          # 16 output graphs per core (host tail bookkeeping)
GRP = B // 4              # 32 groups of 4 graphs packed per 128 partitions
NA = 12                   # adv outputs (3 branches x 4 actions)
NV = 64                   # val hidden
NH = NA + NV              # 76 combined head outputs
NSLAB = 8                 # j-slabs per core
JS = NC_ // NSLAB         # 16 dst nodes per slab
SLABC = JS * GRP * DEG    # 8192 xe cols per slab
BLK = 1024                # PSUM drain block (2 banks)
ASSIST = (1, 3, 5)        # blocks (per slab) drained via ACT + TT-max tree

_CACHE = {}
LAST_RESULTS = None


def _build_program():
    import concourse.bass as bass
    import concourse.bacc as bacc
    import concourse.mybir as mybir
    import concourse.tile as tile

    f32 = mybir.dt.float32
    bf16 = mybir.dt.bfloat16
    Relu = mybir.ActivationFunctionType.Relu
    Alu = mybir.AluOpType

    nc = bacc.Bacc("TRN2", target_bir_lowering=False, debug=False,
                   num_devices=NCORES)

    # ---- kernel I/O ----
    xt_d = nc.declare_dram_parameter("xt", [128, NC_ * GRP], bf16, isOutput=False)
    xe_d = nc.declare_dram_parameter("xe", [128, NSLAB * SLABC], bf16, isOutput=False)
    wpool_d = nc.declare_dram_parameter("wpool_bd", [128, 128], bf16, isOutput=False)
    bpool_d = nc.declare_dram_parameter("bpool", [128, 1], f32, isOutput=False)
    wself_d = nc.declare_dram_parameter("wself_bd", [128, 4 * H], bf16, isOutput=False)
    wneigh_d = nc.declare_dram_parameter("wneigh_bd", [128, 4 * H], bf16, isOutput=False)
    bsage_d = nc.declare_dram_parameter("bsage", [128, 1], f32, isOutput=False)
    whead_d = nc.declare_dram_parameter("whead", [128, NC_ * NH], bf16, isOutput=False)
    hpart_d = nc.declare_dram_parameter("hpart", [128, NH], f32, isOutput=True)

    import os as _os
    _dbg = _os.environ.get("KDBG") == "1"
    if _dbg:
        dbg_aggT_d = nc.declare_dram_parameter("dbg_aggT", [128, NC_ * GRP], bf16, isOutput=True)
        dbg_ht_d = nc.declare_dram_parameter("dbg_ht", [128, NC_ * B], bf16, isOutput=True)

    with tile.TileContext(nc) as tc:
        with (
            tc.tile_pool(name="const", bufs=1) as cpool,
            tc.tile_pool(name="big", bufs=1) as bigpool,
        ):
            # ---- constants (ordered so slab-0 inputs land first) ----
            wpool = cpool.tile([128, 128], bf16)
            nc.scalar.dma_start(out=wpool[:], in_=wpool_d[:])
            bpool = cpool.tile([128, 1], f32)
            nc.scalar.dma_start(out=bpool[:], in_=bpool_d[:])
            xt = cpool.tile([128, NC_ * GRP], bf16)        # [(q,f), (j,grp)]
            wself = cpool.tile([128, 4 * H], bf16)
            wneigh = cpool.tile([128, 4 * H], bf16)
            bsage = cpool.tile([128, 1], f32)
            whead = cpool.tile([128, NC_ * NH], bf16)      # [h, (j, o)]

            ht = bigpool.tile([128, NC_ * B], bf16)        # [h, j*128 + g]  4MB
            aggT = bigpool.tile([128, NC_ * GRP], bf16)    # [(q,f'), j*32+grp] 1MB

            # head psum allocated up-front: accumulates across all slabs
            hd_ps_ctx = tc.tile_pool(name="hd_ps", bufs=1, space="PSUM")
            hd_ps = hd_ps_ctx.__enter__()
            pshead = hd_ps.tile([128, NH], f32)

            with (
                tc.tile_pool(name="xe_sb", bufs=3) as xe_pool,
                tc.tile_pool(name="z_ps", bufs=3, space="PSUM") as z_ps,
                tc.tile_pool(name="zr_sb", bufs=2) as zr_pool,
                tc.tile_pool(name="h_ps", bufs=1, space="PSUM") as h_ps,
            ):
                NB = SLABC // BLK                          # 8 blocks per slab
                NAS = len(ASSIST)                          # assisted blocks/slab
                htv = ht[:].rearrange("p (j grp q) -> p j grp q", grp=GRP, q=4)

                def stage_d_q(sd, q, eng="gpsimd"):
                    # ht slab sd, quadrant q: relu(W_self x + W_neigh agg + b)
                    hp = h_ps.tile([128, JS * GRP], f32, tag="hps")
                    nc.tensor.matmul(
                        out=hp[:],
                        lhsT=wself[:, q * H:(q + 1) * H],
                        rhs=xt[:, sd * JS * GRP:(sd + 1) * JS * GRP],
                        start=True, stop=False)
                    nc.tensor.matmul(
                        out=hp[:],
                        lhsT=wneigh[:, q * H:(q + 1) * H],
                        rhs=aggT[:, sd * JS * GRP:(sd + 1) * JS * GRP],
                        start=False, stop=True)
                    ov = htv[:, sd * JS:(sd + 1) * JS, :, q]
                    iv = hp[:].rearrange("p (j grp) -> p j grp", grp=GRP)
                    if eng == "scalar":
                        nc.scalar.activation(out=ov, in_=iv, func=Relu,
                                             bias=bsage[:])
                    else:   # DVE; gpsimd cannot touch PSUM on real HW
                        nc.vector.tensor_scalar(
                            out=ov, in0=iv, scalar1=bsage[:], scalar2=0.0,
                            op0=Alu.add, op1=Alu.max)

                def stage_e(se):
                    # head accumulation over slab se's nodes
                    for jj in range(JS):
                        j = se * JS + jj
                        nc.tensor.matmul(
                            out=pshead[:],
                            lhsT=ht[:, j * B:(j + 1) * B],
                            rhs=whead[:, j * NH:(j + 1) * NH],
                            start=(j == 0), stop=(j == NC_ - 1),
                        )

                for s in range(NSLAB):
                    # ---- stage A+B: aggT slab = relu(max_d(W_pool@x[src]) + b) ----
                    # xe slab cols: (jj 16, grp 32, d 16); aggT cols: j*32+grp
                    xe = xe_pool.tile([128, SLABC], bf16, tag="xe")
                    nch = 4 if s == 0 else 2
                    for h2 in range(nch):  # chunks so compute starts earlier
                        cw = SLABC // nch
                        nc.sync.dma_start(
                            out=xe[:, h2 * cw:(h2 + 1) * cw],
                            in_=xe_d[:, s * SLABC + h2 * cw:
                                     s * SLABC + (h2 + 1) * cw])
                    if s == 0:
                        # deferred consts on the scalar-triggered DMA ring:
                        # in flight alongside xe slab 0, ready for stage D
                        nc.scalar.dma_start(out=xt[:], in_=xt_d[:])
                        nc.scalar.dma_start(out=wself[:], in_=wself_d[:])
                        nc.scalar.dma_start(out=wneigh[:], in_=wneigh_d[:])
                        nc.scalar.dma_start(out=bsage[:], in_=bsage_d[:])
                    # whead chunk c is first read by E(c) during slab c+2;
                    # issue it one slab late to keep early DMA bw for xe
                    wchunks = [s - 1] if s > 0 else []
                    if s == NSLAB - 1:
                        wchunks.append(s)
                    for wc in wchunks:
                        nc.sync.dma_start(
                            out=whead[:, wc * JS * NH:(wc + 1) * JS * NH],
                            in_=whead_d[:, wc * JS * NH:(wc + 1) * JS * NH])
                    zr = zr_pool.tile([128, NAS * BLK], bf16, tag="zr")
                    a0 = s * JS * GRP                      # aggT col offset
                    for blk in range(NB):   # 8 blocks of 1024 (64 nd, 16 d)
                        ps = z_ps.tile([128, BLK], f32, tag="zps")
                        for h2 in range(2):   # one matmul per PSUM bank
                            nc.tensor.matmul(
                                out=ps[:, h2 * 512:(h2 + 1) * 512],
                                lhsT=wpool[:],
                                rhs=xe[:, blk * BLK + h2 * 512:
                                        blk * BLK + (h2 + 1) * 512],
                                start=True, stop=True,
                            )
                        if blk in ASSIST:
                            # fused relu+bias drain on ACT; max-tree later
                            slot = ASSIST.index(blk)
                            nc.scalar.activation(
                                out=zr[:, slot * BLK:(slot + 1) * BLK],
                                in_=ps[:], func=Relu, bias=bpool[:])
                        else:
                            nc.vector.reduce_max(
                                out=aggT[:, a0 + blk * (BLK // DEG):
                                         a0 + (blk + 1) * (BLK // DEG)],
                                in_=ps[:].rearrange("p (n d) -> p n d", d=DEG),
                                axis=mybir.AxisListType.X)
                        # software-pipelined stage D of the previous slab:
                        # one quadrant between A-blocks keeps PE fed during
                        # D's psum-drain waits
                        if s > 0 and blk in (1, 3, 5, 7):
                            stage_d_q(s - 1, blk // 2, eng="scalar")
                    if s > 1:
                        stage_e(s - 2)   # deferred: whead DMA gets slack
                    # batched TT-max tree over d for assisted blocks (2x_1p)
                    ND = NAS * BLK // DEG                  # tree output cols
                    zrv = zr[:].rearrange("p (n d) -> p n d", d=DEG)
                    t1 = zr_pool.tile([128, ND * 8], bf16, tag="t1")
                    t1v = t1[:].rearrange("p (n d) -> p n d", d=8)
                    nc.vector.tensor_tensor(
                        out=t1v[:], in0=zrv[:, :, 0:8], in1=zrv[:, :, 8:16],
                        op=Alu.max)
                    t2 = zr_pool.tile([128, ND * 4], bf16, tag="t2")
                    t2v = t2[:].rearrange("p (n d) -> p n d", d=4)
                    nc.vector.tensor_tensor(
                        out=t2v[:], in0=t1v[:, :, 0:4], in1=t1v[:, :, 4:8],
                        op=Alu.max)
                    t3 = zr_pool.tile([128, ND * 2], bf16, tag="t3")
                    t3v = t3[:].rearrange("p (n d) -> p n d", d=2)
                    nc.vector.tensor_tensor(
                        out=t3v[:], in0=t2v[:, :, 0:2], in1=t2v[:, :, 2:4],
                        op=Alu.max)
                    # assisted aggT strips {1,3,5}: strided [128, 3, 64]
                    av = aggT[:].rearrange("p (b n) -> p b n", n=BLK // DEG)
                    t4a = t3v[:, :, 0].rearrange("p (b n) -> p b n", b=NAS)
                    t4b = t3v[:, :, 1].rearrange("p (b n) -> p b n", b=NAS)
                    nc.vector.tensor_tensor(
                        out=av[:, NB * s + 1:NB * s + 2 * NAS:2, :],
                        in0=t4a[:], in1=t4b[:], op=Alu.max)
                    # relu+bias for direct strips: {0,2,4} strided + {6,7}
                    nc.vector.tensor_scalar(
                        out=av[:, NB * s:NB * s + 2 * NAS:2, :],
                        in0=av[:, NB * s:NB * s + 2 * NAS:2, :],
                        scalar1=bpool[:], scalar2=0.0,
                        op0=Alu.add, op1=Alu.max)
                    nc.vector.tensor_scalar(
                        out=av[:, NB * s + 2 * NAS:NB * (s + 1), :],
                        in0=av[:, NB * s + 2 * NAS:NB * (s + 1), :],
                        scalar1=bpool[:], scalar2=0.0,
                        op0=Alu.add, op1=Alu.max)

                # drained-epilogue: stage D+E for the last slab; alternate
                # the drains over ACT/DVE since nothing else fills the tail
                for q, eng in enumerate(("scalar", "vector", "scalar", "vector")):
                    stage_d_q(NSLAB - 1, q, eng=eng)
                stage_e(NSLAB - 2)
                stage_e(NSLAB - 1)

            if _dbg:
                nc.sync.dma_start(out=dbg_aggT_d[:], in_=aggT[:])
                nc.sync.dma_start(out=dbg_ht_d[:], in_=ht[:])

            # ---- output per-graph head partials; tail is summed on host ----
            with tc.tile_pool(name="tail", bufs=1) as tp:
                psf = tp.tile([128, NH], f32)
                nc.scalar.copy(out=psf[:], in_=pshead[:])
                nc.sync.dma_start(out=hpart_d[:], in_=psf[:])
            hd_ps_ctx.__exit__(None, None, None)
    nc.compile()
    return nc


def _make_in_maps(inputs):
    import ml_dtypes
    bf = ml_dtypes.bfloat16

    x = np.asarray(inputs["x"], np.float32)
    src = np.asarray(inputs["src"], np.int64)
    W_pool = np.asarray(inputs["W_pool"], np.float32)
    b_pool = np.asarray(inputs["b_pool"], np.float32)
    W_self = np.asarray(inputs["W_self"], np.float32)
    W_neigh = np.asarray(inputs["W_neigh"], np.float32)
    b_sage = np.asarray(inputs["b_sage"], np.float32)
    W_adv = np.asarray(inputs["W_adv"], np.float32)
    W_v1 = np.asarray(inputs["W_v1"], np.float32)

    # shared (replicated) tensors
    wpool_bd = np.kron(np.eye(4, dtype=np.float32), W_pool.T).astype(bf)  # [128,128]
    bpool = np.ascontiguousarray(np.tile(b_pool, 4)[:, None], np.float32)
    wself_bd = np.zeros((128, 4 * H), np.float32)
    wneigh_bd = np.zeros((128, 4 * H), np.float32)
    for q in range(4):
        wself_bd[q * 32:(q + 1) * 32, q * H:(q + 1) * H] = W_self.T
        wneigh_bd[q * 32:(q + 1) * 32, q * H:(q + 1) * H] = W_neigh.T
    bsage = np.ascontiguousarray(b_sage[:, None])
    W_cat = np.concatenate([W_adv, W_v1], axis=0)          # [76, 131072]
    W_cat = W_cat.reshape(NH, N, H)                        # [o, n, h]

    shared = {
        "wpool_bd": wpool_bd, "bpool": bpool,
        "wself_bd": wself_bd.astype(bf), "wneigh_bd": wneigh_bd.astype(bf),
        "bsage": bsage,
    }

    xbf = x.astype(bf)                                     # [128, 1024, 32]
    sidx = (src.reshape(B, N, DEG)
            - (np.arange(B, dtype=np.int64) * N)[:, None, None])  # local [0,N)
    garange = np.arange(B)[:, None, None]

    in_maps = []
    for c in range(NCORES):
        jsl = slice(NC_ * c, NC_ * (c + 1))
        xs = xbf[:, jsl, :]                                # [128g, 128j, 32f]
        # xt[(q,f), j*32+grp] = x[grp*4+q, 128c+j, f]
        xt = np.ascontiguousarray(
            xs.reshape(GRP, 4, NC_, F).transpose(1, 3, 2, 0)
            .reshape(128, NC_ * GRP))
        # xe[(q,f), ((j*32)+grp)*16+d] = x[g, src_local[g, 128c+j, d], f]
        sl = sidx[:, jsl, :]                               # [128g, 128j, 16d]
        xg = xbf[garange, sl, :]                           # [g, j, d, f]
        xe = np.ascontiguousarray(
            xg.reshape(GRP, 4, NC_, DEG, F).transpose(1, 4, 2, 0, 3)
            .reshape(128, NC_ * GRP * DEG))
        # whead[h, j*76+o] = W_cat[o, 128c+j, h]
        whead = np.ascontiguousarray(
            W_cat[:, jsl, :].transpose(2, 1, 0).reshape(H, NC_ * NH)).astype(bf)
        in_maps.append({"xt": xt, "xe": xe, "whead": whead, **shared})
    return in_maps


def _host_tail(hsum, inputs):
    """Dueling tail on the summed head partials [128, 76] (fp32, tiny)."""
    b_adv = np.asarray(inputs["b_adv"], np.float32)
    b_v1 = np.asarray(inputs["b_v1"], np.float32)
    W_v2 = np.asarray(inputs["W_v2"], np.float32)
    b_v2 = np.asarray(inputs["b_v2"], np.float32)
    W_v3 = np.asarray(inputs["W_v3"], np.float32)
    b_v3 = np.asarray(inputs["b_v3"], np.float32)

    adv = np.maximum(hsum[:, :NA] + b_adv, 0.0).reshape(B, 3, 4)
    val = np.maximum(hsum[:, NA:] + b_v1, 0.0)
    val = np.maximum(val @ W_v2.T + b_v2, 0.0)
    val = val @ W_v3.T + b_v3                               # [B, 1]
    return val[..., None] + adv - adv.mean(-1, keepdims=True)


def kernel(**inputs) -> np.ndarray:
    global LAST_RESULTS
    from concourse.bass_utils import run_bass_kernel_spmd

    if "nc" not in _CACHE:
        _CACHE["nc"] = _build_program()
    nc = _CACHE["nc"]
    in_maps = _make_in_maps(inputs)
    rr = run_bass_kernel_spmd(nc, in_maps, list(range(NCORES)))
    LAST_RESULTS = rr
    hsum = np.zeros((B, NH), np.float32)
    for c in range(NCORES):
        hsum += rr.results[c]["hpart"]
    return _host_tail(hsum, inputs).astype(np.float32)


# revision 28
# speedup vs baseline: 1.0334x; 1.0334x over previous
"""Trainium2 Bass kernel for nn_BHS_SAGE (GNN message passing + dueling head).

Node-chunk sharding: core c owns nodes [128c, 128(c+1)) of ALL 128 graphs
(instead of 16 whole graphs).  The SAGE stages (pool-MLP, edge max-agg,
self+neigh matmul) see the same per-core work either way, but the dueling
head only needs this core's 128-node slice of W_adv/W_v1 (2.4 MB instead of
the full 19.9 MB replicated), and its matmuls run at M=128 (full PE rows).

Each core returns its per-graph head partial sums [128 g, 76] in fp32; the
host sums the 8 partials and applies the dueling tail (76 -> 12 outputs per
graph, ~60 KFLOP total vs ~9 GFLOP on device).  An on-device ReduceScatter
was measured at ~50 us of firmware latency for 39 KB and abandoned.

Per-core pipeline, 8 slabs (each slab = 16 dst nodes x 32 graph-groups):
  A. z = W_pool_blockdiag @ xe-slab (4-graph packed, 1024-col PSUM blocks)
  B. agg = relu(max_d z + b); two drain flavours balanced across engines:
       direct slab:   DVE reduce_max from PSUM (1x), bias+relu after (4x TS)
       assisted slab: ACT relu+bias drain PSUM->SBUF bf16, then a
                      slab-batched TT-max tree (2x_1p) on DVE
  D. h = relu(W_self x + W_neigh agg + b) per graph quadrant (ACT drain)
  E. head psum[128 g, 76] += ht[:, j].T @ whead[:, j]  (16 matmuls/slab,
     accumulated across all 128 j; hidden under the slab drains)
"""

import numpy as np

B, N, F, H, DEG = 128, 1024, 32, 128, 16
NCORES = 8
NC_ = N // NCORES         # 128 nodes per core chunk
BL = B // NCORES          # 16 output graphs per core (host tail bookkeeping)
GRP = B // 4              # 32 groups of 4 graphs packed per 128 partitions
NA = 12                   # adv outputs (3 branches x 4 actions)
NV = 64                   # val hidden
NH = NA + NV              # 76 combined head outputs
NSLAB = 8                 # j-slabs per core
JS = NC_ // NSLAB         # 16 dst nodes per slab
SLABC = JS * GRP * DEG    # 8192 xe cols per slab
BLK = 1024                # PSUM drain block (2 banks)
ASSIST = (1, 3, 5)        # blocks (per slab) drained via ACT + TT-max tree

_CACHE = {}
LAST_RESULTS = None


def _build_program():
    import concourse.bass as bass
    import concourse.bacc as bacc
    import concourse.mybir as mybir
    import concourse.tile as tile

    f32 = mybir.dt.float32
    bf16 = mybir.dt.bfloat16
    Relu = mybir.ActivationFunctionType.Relu
    Alu = mybir.AluOpType

    nc = bacc.Bacc("TRN2", target_bir_lowering=False, debug=False,
                   num_devices=NCORES)

    # ---- kernel I/O ----
    xt_d = nc.declare_dram_parameter("xt", [128, NC_ * GRP], bf16, isOutput=False)
    xe_d = nc.declare_dram_parameter("xe", [128, NSLAB * SLABC], bf16, isOutput=False)
    wpool_d = nc.declare_dram_parameter("wpool_bd", [128, 128], bf16, isOutput=False)
    bpool_d = nc.declare_dram_parameter("bpool", [128, 1], f32, isOutput=False)
    wself_d = nc.declare_dram_parameter("wself_bd", [128, 4 * H], bf16, isOutput=False)
    wneigh_d = nc.declare_dram_parameter("wneigh_bd", [128, 4 * H], bf16, isOutput=False)
    bsage_d = nc.declare_dram_parameter("bsage", [128, 1], f32, isOutput=False)
    whead_d = nc.declare_dram_parameter("whead", [128, NC_ * NH], bf16, isOutput=False)
    hpart_d = nc.declare_dram_parameter("hpart", [128, NH], f32, isOutput=True)

    import os as _os
    _dbg = _os.environ.get("KDBG") == "1"
    if _dbg:
        dbg_aggT_d = nc.declare_dram_parameter("dbg_aggT", [128, NC_ * GRP], bf16, isOutput=True)
        dbg_ht_d = nc.declare_dram_parameter("dbg_ht", [128, NC_ * B], bf16, isOutput=True)

    with tile.TileContext(nc) as tc:
        with (
            tc.tile_pool(name="const", bufs=1) as cpool,
            tc.tile_pool(name="big", bufs=1) as bigpool,
        ):
            # ---- constants (ordered so slab-0 inputs land first) ----
            wpool = cpool.tile([128, 128], bf16)
            nc.scalar.dma_start(out=wpool[:], in_=wpool_d[:])
            bpool = cpool.tile([128, 1], f32)
            nc.scalar.dma_start(out=bpool[:], in_=bpool_d[:])
            xt = cpool.tile([128, NC_ * GRP], bf16)        # [(q,f), (j,grp)]
            wself = cpool.tile([128, 4 * H], bf16)
            wneigh = cpool.tile([128, 4 * H], bf16)
            bsage = cpool.tile([128, 1], f32)
            whead = cpool.tile([128, NC_ * NH], bf16)      # [h, (j, o)]

            ht = bigpool.tile([128, NC_ * B], bf16)        # [h, j*128 + g]  4MB
            aggT = bigpool.tile([128, NC_ * GRP], bf16)    # [(q,f'), j*32+grp] 1MB

            # head psum allocated up-front: accumulates across all slabs
            hd_ps_ctx = tc.tile_pool(name="hd_ps", bufs=1, space="PSUM")
            hd_ps = hd_ps_ctx.__enter__()
            pshead = hd_ps.tile([128, NH], f32)

            with (
                tc.tile_pool(name="xe_sb", bufs=3) as xe_pool,
                tc.tile_pool(name="z_ps", bufs=3, space="PSUM") as z_ps,
                tc.tile_pool(name="zr_sb", bufs=2) as zr_pool,
            ):
                NB = SLABC // BLK                          # 8 blocks per slab
                NAS = len(ASSIST)                          # assisted blocks/slab
                htv = ht[:].rearrange("p (j grp q) -> p j grp q", grp=GRP, q=4)

                def stage_d_q(sd, q, eng="scalar"):
                    # ht slab sd, quadrant q: relu(W_self x + W_neigh agg + b)
                    # borrows z_ps ring tiles (first 512 cols) so consecutive
                    # q-iterations pipeline instead of serializing on one tile
                    hpf = z_ps.tile([128, BLK], f32, tag="zps")
                    hp = hpf[:, 0:JS * GRP]
                    nc.tensor.matmul(
                        out=hp,
                        lhsT=wself[:, q * H:(q + 1) * H],
                        rhs=xt[:, sd * JS * GRP:(sd + 1) * JS * GRP],
                        start=True, stop=False)
                    nc.tensor.matmul(
                        out=hp,
                        lhsT=wneigh[:, q * H:(q + 1) * H],
                        rhs=aggT[:, sd * JS * GRP:(sd + 1) * JS * GRP],
                        start=False, stop=True)
                    ov = htv[:, sd * JS:(sd + 1) * JS, :, q]
                    iv = hp.rearrange("p (j grp) -> p j grp", grp=GRP)
                    if eng == "scalar":
                        nc.scalar.activation(out=ov, in_=iv, func=Relu,
                                             bias=bsage[:])
                    else:   # DVE; gpsimd cannot touch PSUM on real HW
                        nc.vector.tensor_scalar(
                            out=ov, in0=iv, scalar1=bsage[:], scalar2=0.0,
                            op0=Alu.add, op1=Alu.max)

                def stage_e(se):
                    # head accumulation over slab se's nodes
                    for jj in range(JS):
                        j = se * JS + jj
                        nc.tensor.matmul(
                            out=pshead[:],
                            lhsT=ht[:, j * B:(j + 1) * B],
                            rhs=whead[:, j * NH:(j + 1) * NH],
                            start=(j == 0), stop=(j == NC_ - 1),
                        )

                for s in range(NSLAB):
                    # ---- stage A+B: aggT slab = relu(max_d(W_pool@x[src]) + b) ----
                    # xe slab cols: (jj 16, grp 32, d 16); aggT cols: j*32+grp
                    xe = xe_pool.tile([128, SLABC], bf16, tag="xe")
                    nch = 4 if s == 0 else 2
                    for h2 in range(nch):  # chunks so compute starts earlier
                        cw = SLABC // nch
                        nc.sync.dma_start(
                            out=xe[:, h2 * cw:(h2 + 1) * cw],
                            in_=xe_d[:, s * SLABC + h2 * cw:
                                     s * SLABC + (h2 + 1) * cw])
                    if s == 0:
                        # deferred consts on the scalar-triggered DMA ring:
                        # in flight alongside xe slab 0, ready for stage D
                        nc.scalar.dma_start(out=xt[:], in_=xt_d[:])
                        nc.scalar.dma_start(out=wself[:], in_=wself_d[:])
                        nc.scalar.dma_start(out=wneigh[:], in_=wneigh_d[:])
                        nc.scalar.dma_start(out=bsage[:], in_=bsage_d[:])
                    nc.sync.dma_start(
                        out=whead[:, s * JS * NH:(s + 1) * JS * NH],
                        in_=whead_d[:, s * JS * NH:(s + 1) * JS * NH])
                    zr = zr_pool.tile([128, NAS * BLK], bf16, tag="zr")
                    a0 = s * JS * GRP                      # aggT col offset
                    for blk in range(NB):   # 8 blocks of 1024 (64 nd, 16 d)
                        ps = z_ps.tile([128, BLK], f32, tag="zps")
                        for h2 in range(2):   # one matmul per PSUM bank
                            nc.tensor.matmul(
                                out=ps[:, h2 * 512:(h2 + 1) * 512],
                                lhsT=wpool[:],
                                rhs=xe[:, blk * BLK + h2 * 512:
                                        blk * BLK + (h2 + 1) * 512],
                                start=True, stop=True,
                            )
                        if blk in ASSIST:
                            # fused relu+bias drain on ACT; max-tree later
                            slot = ASSIST.index(blk)
                            nc.scalar.activation(
                                out=zr[:, slot * BLK:(slot + 1) * BLK],
                                in_=ps[:], func=Relu, bias=bpool[:])
                        else:
                            nc.vector.reduce_max(
                                out=aggT[:, a0 + blk * (BLK // DEG):
                                         a0 + (blk + 1) * (BLK // DEG)],
                                in_=ps[:].rearrange("p (n d) -> p n d", d=DEG),
                                axis=mybir.AxisListType.X)

                    # batched TT-max tree over d for assisted blocks (2x_1p)
                    ND = NAS * BLK // DEG                  # tree output cols
                    zrv = zr[:].rearrange("p (n d) -> p n d", d=DEG)
                    t1 = zr_pool.tile([128, ND * 8], bf16, tag="t1")
                    t1v = t1[:].rearrange("p (n d) -> p n d", d=8)
                    nc.vector.tensor_tensor(
                        out=t1v[:], in0=zrv[:, :, 0:8], in1=zrv[:, :, 8:16],
                        op=Alu.max)
                    t2 = zr_pool.tile([128, ND * 4], bf16, tag="t2")
                    t2v = t2[:].rearrange("p (n d) -> p n d", d=4)
                    nc.vector.tensor_tensor(
                        out=t2v[:], in0=t1v[:, :, 0:4], in1=t1v[:, :, 4:8],
                        op=Alu.max)
                    t3 = zr_pool.tile([128, ND * 2], bf16, tag="t3")
                    t3v = t3[:].rearrange("p (n d) -> p n d", d=2)
                    nc.vector.tensor_tensor(
                        out=t3v[:], in0=t2v[:, :, 0:2], in1=t2v[:, :, 2:4],
                        op=Alu.max)
                    # assisted aggT strips {1,3,5}: strided [128, 3, 64]
                    av = aggT[:].rearrange("p (b n) -> p b n", n=BLK // DEG)
                    t4a = t3v[:, :, 0].rearrange("p (b n) -> p b n", b=NAS)
                    t4b = t3v[:, :, 1].rearrange("p (b n) -> p b n", b=NAS)
                    nc.vector.tensor_tensor(
                        out=av[:, NB * s + 1:NB * s + 2 * NAS:2, :],
                        in0=t4a[:], in1=t4b[:], op=Alu.max)
                    # relu+bias for direct strips: {0,2,4} strided + {6,7}
                    nc.vector.tensor_scalar(
                        out=av[:, NB * s:NB * s + 2 * NAS:2, :],
                        in0=av[:, NB * s:NB * s + 2 * NAS:2, :],
                        scalar1=bpool[:], scalar2=0.0,
                        op0=Alu.add, op1=Alu.max)
                    nc.vector.tensor_scalar(
                        out=av[:, NB * s + 2 * NAS:NB * (s + 1), :],
                        in0=av[:, NB * s + 2 * NAS:NB * (s + 1), :],
                        scalar1=bpool[:], scalar2=0.0,
                        op0=Alu.add, op1=Alu.max)

                    # ---- stage D + E for this slab ----
                    # last slab: alternate drains over ACT/DVE (bare tail)
                    engs = (("scalar", "vector", "scalar", "vector")
                            if s == NSLAB - 1 else ("scalar",) * 4)
                    for q in range(4):
                        stage_d_q(s, q, eng=engs[q])
                    stage_e(s)

            if _dbg:
                nc.sync.dma_start(out=dbg_aggT_d[:], in_=aggT[:])
                nc.sync.dma_start(out=dbg_ht_d[:], in_=ht[:])

            # ---- output per-graph head partials; tail is summed on host ----
            with tc.tile_pool(name="tail", bufs=1) as tp:
                psf = tp.tile([128, NH], f32)
                nc.scalar.copy(out=psf[:], in_=pshead[:])
                nc.sync.dma_start(out=hpart_d[:], in_=psf[:])
            hd_ps_ctx.__exit__(None, None, None)
    nc.compile()
    return nc


def _make_in_maps(inputs):
    import ml_dtypes
    bf = ml_dtypes.bfloat16

    x = np.asarray(inputs["x"], np.float32)
    src = np.asarray(inputs["src"], np.int64)
    W_pool = np.asarray(inputs["W_pool"], np.float32)
    b_pool = np.asarray(inputs["b_pool"], np.float32)
    W_self = np.asarray(inputs["W_self"], np.float32)
    W_neigh = np.asarray(inputs["W_neigh"], np.float32)
    b_sage = np.asarray(inputs["b_sage"], np.float32)
    W_adv = np.asarray(inputs["W_adv"], np.float32)
    W_v1 = np.asarray(inputs["W_v1"], np.float32)

    # shared (replicated) tensors
    wpool_bd = np.kron(np.eye(4, dtype=np.float32), W_pool.T).astype(bf)  # [128,128]
    bpool = np.ascontiguousarray(np.tile(b_pool, 4)[:, None], np.float32)
    wself_bd = np.zeros((128, 4 * H), np.float32)
    wneigh_bd = np.zeros((128, 4 * H), np.float32)
    for q in range(4):
        wself_bd[q * 32:(q + 1) * 32, q * H:(q + 1) * H] = W_self.T
        wneigh_bd[q * 32:(q + 1) * 32, q * H:(q + 1) * H] = W_neigh.T
    bsage = np.ascontiguousarray(b_sage[:, None])
    W_cat = np.concatenate([W_adv, W_v1], axis=0)          # [76, 131072]
    W_cat = W_cat.reshape(NH, N, H)                        # [o, n, h]

    shared = {
        "wpool_bd": wpool_bd, "bpool": bpool,
        "wself_bd": wself_bd.astype(bf), "wneigh_bd": wneigh_bd.astype(bf),
        "bsage": bsage,
    }

    xbf = x.astype(bf)                                     # [128, 1024, 32]
    sidx = (src.reshape(B, N, DEG)
            - (np.arange(B, dtype=np.int64) * N)[:, None, None])  # local [0,N)
    garange = np.arange(B)[:, None, None]

    in_maps = []
    for c in range(NCORES):
        jsl = slice(NC_ * c, NC_ * (c + 1))
        xs = xbf[:, jsl, :]                                # [128g, 128j, 32f]
        # xt[(q,f), j*32+grp] = x[grp*4+q, 128c+j, f]
        xt = np.ascontiguousarray(
            xs.reshape(GRP, 4, NC_, F).transpose(1, 3, 2, 0)
            .reshape(128, NC_ * GRP))
        # xe[(q,f), ((j*32)+grp)*16+d] = x[g, src_local[g, 128c+j, d], f]
        sl = sidx[:, jsl, :]                               # [128g, 128j, 16d]
        xg = xbf[garange, sl, :]                           # [g, j, d, f]
        xe = np.ascontiguousarray(
            xg.reshape(GRP, 4, NC_, DEG, F).transpose(1, 4, 2, 0, 3)
            .reshape(128, NC_ * GRP * DEG))
        # whead[h, j*76+o] = W_cat[o, 128c+j, h]
        whead = np.ascontiguousarray(
            W_cat[:, jsl, :].transpose(2, 1, 0).reshape(H, NC_ * NH)).astype(bf)
        in_maps.append({"xt": xt, "xe": xe, "whead": whead, **shared})
    return in_maps


def _host_tail(hsum, inputs):
    """Dueling tail on the summed head partials [128, 76] (fp32, tiny)."""
    b_adv = np.asarray(inputs["b_adv"], np.float32)
    b_v1 = np.asarray(inputs["b_v1"], np.float32)
    W_v2 = np.asarray(inputs["W_v2"], np.float32)
    b_v2 = np.asarray(inputs["b_v2"], np.float32)
    W_v3 = np.asarray(inputs["W_v3"], np.float32)
    b_v3 = np.asarray(inputs["b_v3"], np.float32)

    adv = np.maximum(hsum[:, :NA] + b_adv, 0.0).reshape(B, 3, 4)
    val = np.maximum(hsum[:, NA:] + b_v1, 0.0)
    val = np.maximum(val @ W_v2.T + b_v2, 0.0)
    val = val @ W_v3.T + b_v3                               # [B, 1]
    return val[..., None] + adv - adv.mean(-1, keepdims=True)


def kernel(**inputs) -> np.ndarray:
    global LAST_RESULTS
    from concourse.bass_utils import run_bass_kernel_spmd

    if "nc" not in _CACHE:
        _CACHE["nc"] = _build_program()
    nc = _CACHE["nc"]
    in_maps = _make_in_maps(inputs)
    rr = run_bass_kernel_spmd(nc, in_maps, list(range(NCORES)))
    LAST_RESULTS = rr
    hsum = np.zeros((B, NH), np.float32)
    for c in range(NCORES):
        hsum += rr.results[c]["hpart"]
    return _host_tail(hsum, inputs).astype(np.float32)


# revision 32
# speedup vs baseline: 1.2710x; 1.2298x over previous
"""Trainium2 Bass kernel for nn_BHS_SAGE (GNN message passing + dueling head).

Node-chunk sharding: core c owns nodes [128c, 128(c+1)) of ALL 128 graphs
(instead of 16 whole graphs).  The SAGE stages (pool-MLP, edge max-agg,
self+neigh matmul) see the same per-core work either way, but the dueling
head only needs this core's 128-node slice of W_adv/W_v1 (2.4 MB instead of
the full 19.9 MB replicated), and its matmuls run at M=128 (full PE rows).

Each core returns its per-graph head partial sums [128 g, 76] in fp32; the
host sums the 8 partials and applies the dueling tail (76 -> 12 outputs per
graph, ~60 KFLOP total vs ~9 GFLOP on device).  An on-device ReduceScatter
was measured at ~50 us of firmware latency for 39 KB and abandoned.

Per-core pipeline, 8 slabs (each slab = 16 dst nodes x 32 graph-groups):
  A. z = W_pool_blockdiag @ xe-slab (4-graph packed, 1024-col PSUM blocks)
  B. agg = relu(max_d z + b); two drain flavours balanced across engines:
       direct slab:   DVE reduce_max from PSUM (1x), bias+relu after (4x TS)
       assisted slab: ACT relu+bias drain PSUM->SBUF bf16, then a
                      slab-batched TT-max tree (2x_1p) on DVE
  D. h = relu(W_self x + W_neigh agg + b) per graph quadrant (ACT drain)
  E. head psum[128 g, 76] += ht[:, j].T @ whead[:, j]  (16 matmuls/slab,
     accumulated across all 128 j; hidden under the slab drains)
"""

import numpy as np

B, N, F, H, DEG = 128, 1024, 32, 128, 16
NCORES = 8
NC_ = N // NCORES         # 128 nodes per core chunk
BL = B // NCORES          # 16 output graphs per core (host tail bookkeeping)
GRP = B // 4              # 32 groups of 4 graphs packed per 128 partitions
NA = 12                   # adv outputs (3 branches x 4 actions)
NV = 64                   # val hidden
NH = NA + NV              # 76 combined head outputs
NSLAB = 8                 # j-slabs per core
JS = NC_ // NSLAB         # 16 dst nodes per slab
SLABC = JS * GRP * DEG    # 8192 xe cols per slab
BLK = 1024                # PSUM drain block (2 banks)
ASSIST = (1, 3, 5)        # blocks (per slab) drained via ACT + TT-max tree

_CACHE = {}
LAST_RESULTS = None


def _build_program():
    import concourse.bass as bass
    import concourse.bacc as bacc
    import concourse.mybir as mybir
    import concourse.tile as tile

    f32 = mybir.dt.float32
    bf16 = mybir.dt.bfloat16
    Relu = mybir.ActivationFunctionType.Relu
    Alu = mybir.AluOpType

    nc = bacc.Bacc("TRN2", target_bir_lowering=False, debug=False,
                   num_devices=NCORES)

    # ---- kernel I/O ----
    xt_d = nc.declare_dram_parameter("xt", [128, NC_ * GRP], bf16, isOutput=False)
    xe_d = nc.declare_dram_parameter("xe", [128, NSLAB * SLABC], bf16, isOutput=False)
    wpool_d = nc.declare_dram_parameter("wpool_bd", [128, 128], bf16, isOutput=False)
    bpool_d = nc.declare_dram_parameter("bpool", [128, 1], f32, isOutput=False)
    wself_d = nc.declare_dram_parameter("wself_bd", [128, 4 * H], bf16, isOutput=False)
    wneigh_d = nc.declare_dram_parameter("wneigh_bd", [128, 4 * H], bf16, isOutput=False)
    bsage_d = nc.declare_dram_parameter("bsage", [128, 1], f32, isOutput=False)
    whead_d = nc.declare_dram_parameter("whead", [128, NC_ * NH], bf16, isOutput=False)
    hpart_d = nc.declare_dram_parameter("hpart", [128, NH], f32, isOutput=True)

    import os as _os
    _dbg = _os.environ.get("KDBG") == "1"
    if _dbg:
        dbg_aggT_d = nc.declare_dram_parameter("dbg_aggT", [128, NC_ * GRP], bf16, isOutput=True)
        dbg_ht_d = nc.declare_dram_parameter("dbg_ht", [128, NC_ * B], bf16, isOutput=True)

    with tile.TileContext(nc) as tc:
        with (
            tc.tile_pool(name="const", bufs=1) as cpool,
            tc.tile_pool(name="big", bufs=1) as bigpool,
        ):
            # ---- constants (ordered so slab-0 inputs land first) ----
            wpool = cpool.tile([128, 128], bf16)
            nc.scalar.dma_start(out=wpool[:], in_=wpool_d[:])
            bpool = cpool.tile([128, 1], f32)
            nc.scalar.dma_start(out=bpool[:], in_=bpool_d[:])
            xt = cpool.tile([128, NC_ * GRP], bf16)        # [(q,f), (j,grp)]
            wself = cpool.tile([128, 4 * H], bf16)
            wneigh = cpool.tile([128, 4 * H], bf16)
            bsage = cpool.tile([128, 1], f32)
            whead = cpool.tile([128, NC_ * NH], bf16)      # [h, (j, o)]

            ht = bigpool.tile([128, NC_ * B], bf16)        # [h, j*128 + g]  4MB
            aggT = bigpool.tile([128, NC_ * GRP], bf16)    # [(q,f'), j*32+grp] 1MB

            # head psum allocated up-front: accumulates across all slabs
            hd_ps_ctx = tc.tile_pool(name="hd_ps", bufs=1, space="PSUM")
            hd_ps = hd_ps_ctx.__enter__()
            pshead = hd_ps.tile([128, NH], f32)

            with (
                tc.tile_pool(name="xe_sb", bufs=3) as xe_pool,
                tc.tile_pool(name="z_ps", bufs=3, space="PSUM") as z_ps,
                tc.tile_pool(name="zr_sb", bufs=2) as zr_pool,
                tc.tile_pool(name="h_ps", bufs=1, space="PSUM") as h_ps,
            ):
                NB = SLABC // BLK                          # 8 blocks per slab
                NAS = len(ASSIST)                          # assisted blocks/slab
                htv = ht[:].rearrange("p (j grp q) -> p j grp q", grp=GRP, q=4)

                def stage_d_q(sd, q, eng="scalar", borrow=False):
                    # ht slab sd, quadrant q: relu(W_self x + W_neigh agg + b)
                    # last slab borrows z_ps ring tiles (stage A done) so its
                    # q-iterations pipeline instead of serializing on one tile
                    if borrow:
                        hpt = z_ps.tile([128, BLK], f32, tag="zps")
                        hp = hpt[:, 0:JS * GRP]
                    else:
                        hpt = h_ps.tile([128, JS * GRP], f32, tag="hps")
                        hp = hpt[:]
                    nc.tensor.matmul(
                        out=hp,
                        lhsT=wself[:, q * H:(q + 1) * H],
                        rhs=xt[:, sd * JS * GRP:(sd + 1) * JS * GRP],
                        start=True, stop=False)
                    nc.tensor.matmul(
                        out=hp,
                        lhsT=wneigh[:, q * H:(q + 1) * H],
                        rhs=aggT[:, sd * JS * GRP:(sd + 1) * JS * GRP],
                        start=False, stop=True)
                    ov = htv[:, sd * JS:(sd + 1) * JS, :, q]
                    iv = hp.rearrange("p (j grp) -> p j grp", grp=GRP)
                    if eng == "scalar":
                        nc.scalar.activation(out=ov, in_=iv, func=Relu,
                                             bias=bsage[:])
                    else:   # DVE; gpsimd cannot touch PSUM on real HW
                        nc.vector.tensor_scalar(
                            out=ov, in0=iv, scalar1=bsage[:], scalar2=0.0,
                            op0=Alu.add, op1=Alu.max)

                def stage_e(se):
                    # head accumulation over slab se's nodes
                    for jj in range(JS):
                        j = se * JS + jj
                        nc.tensor.matmul(
                            out=pshead[:],
                            lhsT=ht[:, j * B:(j + 1) * B],
                            rhs=whead[:, j * NH:(j + 1) * NH],
                            start=(j == 0), stop=(j == NC_ - 1),
                        )

                for s in range(NSLAB):
                    # ---- stage A+B: aggT slab = relu(max_d(W_pool@x[src]) + b) ----
                    # xe slab cols: (jj 16, grp 32, d 16); aggT cols: j*32+grp
                    xe = xe_pool.tile([128, SLABC], bf16, tag="xe")
                    nch = 4 if s == 0 else 2
                    for h2 in range(nch):  # chunks so compute starts earlier
                        cw = SLABC // nch
                        nc.sync.dma_start(
                            out=xe[:, h2 * cw:(h2 + 1) * cw],
                            in_=xe_d[:, s * SLABC + h2 * cw:
                                     s * SLABC + (h2 + 1) * cw])
                    if s == 0:
                        # deferred consts on the scalar-triggered DMA ring:
                        # in flight alongside xe slab 0, ready for stage D
                        nc.scalar.dma_start(out=xt[:], in_=xt_d[:])
                        nc.scalar.dma_start(out=wself[:], in_=wself_d[:])
                        nc.scalar.dma_start(out=wneigh[:], in_=wneigh_d[:])
                        nc.scalar.dma_start(out=bsage[:], in_=bsage_d[:])
                    nc.sync.dma_start(
                        out=whead[:, s * JS * NH:(s + 1) * JS * NH],
                        in_=whead_d[:, s * JS * NH:(s + 1) * JS * NH])
                    zr = zr_pool.tile([128, NAS * BLK], bf16, tag="zr")
                    a0 = s * JS * GRP                      # aggT col offset
                    for blk in range(NB):   # 8 blocks of 1024 (64 nd, 16 d)
                        ps = z_ps.tile([128, BLK], f32, tag="zps")
                        for h2 in range(2):   # one matmul per PSUM bank
                            nc.tensor.matmul(
                                out=ps[:, h2 * 512:(h2 + 1) * 512],
                                lhsT=wpool[:],
                                rhs=xe[:, blk * BLK + h2 * 512:
                                        blk * BLK + (h2 + 1) * 512],
                                start=True, stop=True,
                            )
                        if blk in ASSIST:
                            # fused relu+bias drain on ACT; max-tree later
                            slot = ASSIST.index(blk)
                            nc.scalar.activation(
                                out=zr[:, slot * BLK:(slot + 1) * BLK],
                                in_=ps[:], func=Relu, bias=bpool[:])
                        else:
                            nc.vector.reduce_max(
                                out=aggT[:, a0 + blk * (BLK // DEG):
                                         a0 + (blk + 1) * (BLK // DEG)],
                                in_=ps[:].rearrange("p (n d) -> p n d", d=DEG),
                                axis=mybir.AxisListType.X)

                    # batched TT-max tree over d for assisted blocks (2x_1p)
                    ND = NAS * BLK // DEG                  # tree output cols
                    zrv = zr[:].rearrange("p (n d) -> p n d", d=DEG)
                    t1 = zr_pool.tile([128, ND * 8], bf16, tag="t1")
                    t1v = t1[:].rearrange("p (n d) -> p n d", d=8)
                    nc.vector.tensor_tensor(
                        out=t1v[:], in0=zrv[:, :, 0:8], in1=zrv[:, :, 8:16],
                        op=Alu.max)
                    t2 = zr_pool.tile([128, ND * 4], bf16, tag="t2")
                    t2v = t2[:].rearrange("p (n d) -> p n d", d=4)
                    nc.vector.tensor_tensor(
                        out=t2v[:], in0=t1v[:, :, 0:4], in1=t1v[:, :, 4:8],
                        op=Alu.max)
                    t3 = zr_pool.tile([128, ND * 2], bf16, tag="t3")
                    t3v = t3[:].rearrange("p (n d) -> p n d", d=2)
                    nc.vector.tensor_tensor(
                        out=t3v[:], in0=t2v[:, :, 0:2], in1=t2v[:, :, 2:4],
                        op=Alu.max)
                    # assisted aggT strips {1,3,5}: strided [128, 3, 64]
                    av = aggT[:].rearrange("p (b n) -> p b n", n=BLK // DEG)
                    t4a = t3v[:, :, 0].rearrange("p (b n) -> p b n", b=NAS)
                    t4b = t3v[:, :, 1].rearrange("p (b n) -> p b n", b=NAS)
                    nc.vector.tensor_tensor(
                        out=av[:, NB * s + 1:NB * s + 2 * NAS:2, :],
                        in0=t4a[:], in1=t4b[:], op=Alu.max)
                    # relu+bias for direct strips: {0,2,4} strided + {6,7}
                    nc.vector.tensor_scalar(
                        out=av[:, NB * s:NB * s + 2 * NAS:2, :],
                        in0=av[:, NB * s:NB * s + 2 * NAS:2, :],
                        scalar1=bpool[:], scalar2=0.0,
                        op0=Alu.add, op1=Alu.max)
                    nc.vector.tensor_scalar(
                        out=av[:, NB * s + 2 * NAS:NB * (s + 1), :],
                        in0=av[:, NB * s + 2 * NAS:NB * (s + 1), :],
                        scalar1=bpool[:], scalar2=0.0,
                        op0=Alu.add, op1=Alu.max)

                    # ---- stage D + E for this slab ----
                    # last slab: alternate drains over ACT/DVE (bare tail)
                    last = s == NSLAB - 1
                    engs = (("scalar", "vector", "scalar", "vector")
                            if last else ("scalar",) * 4)
                    for q in range(4):
                        stage_d_q(s, q, eng=engs[q], borrow=last)
                    stage_e(s)

            if _dbg:
                nc.sync.dma_start(out=dbg_aggT_d[:], in_=aggT[:])
                nc.sync.dma_start(out=dbg_ht_d[:], in_=ht[:])

            # ---- output per-graph head partials; tail is summed on host ----
            with tc.tile_pool(name="tail", bufs=1) as tp:
                psf = tp.tile([128, NH], f32)
                nc.scalar.copy(out=psf[:], in_=pshead[:])
                nc.sync.dma_start(out=hpart_d[:], in_=psf[:])
            hd_ps_ctx.__exit__(None, None, None)
    nc.compile()
    return nc


def _make_in_maps(inputs):
    import ml_dtypes
    bf = ml_dtypes.bfloat16

    x = np.asarray(inputs["x"], np.float32)
    src = np.asarray(inputs["src"], np.int64)
    W_pool = np.asarray(inputs["W_pool"], np.float32)
    b_pool = np.asarray(inputs["b_pool"], np.float32)
    W_self = np.asarray(inputs["W_self"], np.float32)
    W_neigh = np.asarray(inputs["W_neigh"], np.float32)
    b_sage = np.asarray(inputs["b_sage"], np.float32)
    W_adv = np.asarray(inputs["W_adv"], np.float32)
    W_v1 = np.asarray(inputs["W_v1"], np.float32)

    # shared (replicated) tensors
    wpool_bd = np.kron(np.eye(4, dtype=np.float32), W_pool.T).astype(bf)  # [128,128]
    bpool = np.ascontiguousarray(np.tile(b_pool, 4)[:, None], np.float32)
    wself_bd = np.zeros((128, 4 * H), np.float32)
    wneigh_bd = np.zeros((128, 4 * H), np.float32)
    for q in range(4):
        wself_bd[q * 32:(q + 1) * 32, q * H:(q + 1) * H] = W_self.T
        wneigh_bd[q * 32:(q + 1) * 32, q * H:(q + 1) * H] = W_neigh.T
    bsage = np.ascontiguousarray(b_sage[:, None])
    W_cat = np.concatenate([W_adv, W_v1], axis=0)          # [76, 131072]
    W_cat = W_cat.reshape(NH, N, H)                        # [o, n, h]

    shared = {
        "wpool_bd": wpool_bd, "bpool": bpool,
        "wself_bd": wself_bd.astype(bf), "wneigh_bd": wneigh_bd.astype(bf),
        "bsage": bsage,
    }

    xbf = x.astype(bf)                                     # [128, 1024, 32]
    sidx = (src.reshape(B, N, DEG)
            - (np.arange(B, dtype=np.int64) * N)[:, None, None])  # local [0,N)
    garange = np.arange(B)[:, None, None]

    in_maps = []
    for c in range(NCORES):
        jsl = slice(NC_ * c, NC_ * (c + 1))
        xs = xbf[:, jsl, :]                                # [128g, 128j, 32f]
        # xt[(q,f), j*32+grp] = x[grp*4+q, 128c+j, f]
        xt = np.ascontiguousarray(
            xs.reshape(GRP, 4, NC_, F).transpose(1, 3, 2, 0)
            .reshape(128, NC_ * GRP))
        # xe[(q,f), ((j*32)+grp)*16+d] = x[g, src_local[g, 128c+j, d], f]
        sl = sidx[:, jsl, :]                               # [128g, 128j, 16d]
        xg = xbf[garange, sl, :]                           # [g, j, d, f]
        xe = np.ascontiguousarray(
            xg.reshape(GRP, 4, NC_, DEG, F).transpose(1, 4, 2, 0, 3)
            .reshape(128, NC_ * GRP * DEG))
        # whead[h, j*76+o] = W_cat[o, 128c+j, h]
        whead = np.ascontiguousarray(
            W_cat[:, jsl, :].transpose(2, 1, 0).reshape(H, NC_ * NH)).astype(bf)
        in_maps.append({"xt": xt, "xe": xe, "whead": whead, **shared})
    return in_maps


def _host_tail(hsum, inputs):
    """Dueling tail on the summed head partials [128, 76] (fp32, tiny)."""
    b_adv = np.asarray(inputs["b_adv"], np.float32)
    b_v1 = np.asarray(inputs["b_v1"], np.float32)
    W_v2 = np.asarray(inputs["W_v2"], np.float32)
    b_v2 = np.asarray(inputs["b_v2"], np.float32)
    W_v3 = np.asarray(inputs["W_v3"], np.float32)
    b_v3 = np.asarray(inputs["b_v3"], np.float32)

    adv = np.maximum(hsum[:, :NA] + b_adv, 0.0).reshape(B, 3, 4)
    val = np.maximum(hsum[:, NA:] + b_v1, 0.0)
    val = np.maximum(val @ W_v2.T + b_v2, 0.0)
    val = val @ W_v3.T + b_v3                               # [B, 1]
    return val[..., None] + adv - adv.mean(-1, keepdims=True)


def kernel(**inputs) -> np.ndarray:
    global LAST_RESULTS
    from concourse.bass_utils import run_bass_kernel_spmd

    if "nc" not in _CACHE:
        _CACHE["nc"] = _build_program()
    nc = _CACHE["nc"]
    in_maps = _make_in_maps(inputs)
    rr = run_bass_kernel_spmd(nc, in_maps, list(range(NCORES)))
    LAST_RESULTS = rr
    hsum = np.zeros((B, NH), np.float32)
    for c in range(NCORES):
        hsum += rr.results[c]["hpart"]
    return _host_tail(hsum, inputs).astype(np.float32)


# revision 35
# speedup vs baseline: 1.2848x; 1.0109x over previous
"""Trainium2 Bass kernel for nn_BHS_SAGE (GNN message passing + dueling head).

Node-chunk sharding: core c owns nodes [128c, 128(c+1)) of ALL 128 graphs
(instead of 16 whole graphs).  The SAGE stages (pool-MLP, edge max-agg,
self+neigh matmul) see the same per-core work either way, but the dueling
head only needs this core's 128-node slice of W_adv/W_v1 (2.4 MB instead of
the full 19.9 MB replicated), and its matmuls run at M=128 (full PE rows).

Each core returns its per-graph head partial sums [128 g, 76] in fp32; the
host sums the 8 partials and applies the dueling tail (76 -> 12 outputs per
graph, ~60 KFLOP total vs ~9 GFLOP on device).  An on-device ReduceScatter
was measured at ~50 us of firmware latency for 39 KB and abandoned.

Per-core pipeline, 8 slabs (each slab = 16 dst nodes x 32 graph-groups):
  A. z = W_pool_blockdiag @ xe-slab (4-graph packed, 1024-col PSUM blocks)
  B. agg = relu(max_d z + b); two drain flavours balanced across engines:
       direct slab:   DVE reduce_max from PSUM (1x), bias+relu after (4x TS)
       assisted slab: ACT relu+bias drain PSUM->SBUF bf16, then a
                      slab-batched TT-max tree (2x_1p) on DVE
  D. h = relu(W_self x + W_neigh agg + b) per graph quadrant (ACT drain)
  E. head psum[128 g, 76] += ht[:, j].T @ whead[:, j]  (16 matmuls/slab,
     accumulated across all 128 j; hidden under the slab drains)
"""

import numpy as np

B, N, F, H, DEG = 128, 1024, 32, 128, 16
NCORES = 8
NC_ = N // NCORES         # 128 nodes per core chunk
BL = B // NCORES          # 16 output graphs per core (host tail bookkeeping)
GRP = B // 4              # 32 groups of 4 graphs packed per 128 partitions
NA = 12                   # adv outputs (3 branches x 4 actions)
NV = 64                   # val hidden
NH = NA + NV              # 76 combined head outputs
NSLAB = 8                 # j-slabs per core
JS = NC_ // NSLAB         # 16 dst nodes per slab
SLABC = JS * GRP * DEG    # 8192 xe cols per slab
BLK = 1024                # PSUM drain block (2 banks)
ASSIST = (1, 3, 5)        # blocks (per slab) drained via ACT + TT-max tree

_CACHE = {}
LAST_RESULTS = None


def _build_program():
    import concourse.bass as bass
    import concourse.bacc as bacc
    import concourse.mybir as mybir
    import concourse.tile as tile

    f32 = mybir.dt.float32
    bf16 = mybir.dt.bfloat16
    Relu = mybir.ActivationFunctionType.Relu
    Alu = mybir.AluOpType

    nc = bacc.Bacc("TRN2", target_bir_lowering=False, debug=False,
                   num_devices=NCORES)

    # ---- kernel I/O ----
    xt_d = nc.declare_dram_parameter("xt", [128, NC_ * GRP], bf16, isOutput=False)
    xe_d = nc.declare_dram_parameter("xe", [128, NSLAB * SLABC], bf16, isOutput=False)
    wpool_d = nc.declare_dram_parameter("wpool_bd", [128, 128], bf16, isOutput=False)
    bpool_d = nc.declare_dram_parameter("bpool", [128, 1], f32, isOutput=False)
    wself_d = nc.declare_dram_parameter("wself_bd", [128, 4 * H], bf16, isOutput=False)
    wneigh_d = nc.declare_dram_parameter("wneigh_bd", [128, 4 * H], bf16, isOutput=False)
    bsage_d = nc.declare_dram_parameter("bsage", [128, 1], f32, isOutput=False)
    whead_d = nc.declare_dram_parameter("whead", [128, NC_ * NH], bf16, isOutput=False)
    hpart_d = nc.declare_dram_parameter("hpart", [128, NH], f32, isOutput=True)

    import os as _os
    _dbg = _os.environ.get("KDBG") == "1"
    if _dbg:
        dbg_aggT_d = nc.declare_dram_parameter("dbg_aggT", [128, NC_ * GRP], bf16, isOutput=True)
        dbg_ht_d = nc.declare_dram_parameter("dbg_ht", [128, NC_ * B], bf16, isOutput=True)

    with tile.TileContext(nc) as tc:
        with (
            tc.tile_pool(name="const", bufs=1) as cpool,
            tc.tile_pool(name="big", bufs=1) as bigpool,
        ):
            # ---- constants (ordered so slab-0 inputs land first) ----
            wpool = cpool.tile([128, 128], bf16)
            nc.scalar.dma_start(out=wpool[:], in_=wpool_d[:])
            bpool = cpool.tile([128, 1], f32)
            nc.scalar.dma_start(out=bpool[:], in_=bpool_d[:])
            xt = cpool.tile([128, NC_ * GRP], bf16)        # [(q,f), (j,grp)]
            wself = cpool.tile([128, 4 * H], bf16)
            wneigh = cpool.tile([128, 4 * H], bf16)
            bsage = cpool.tile([128, 1], f32)
            whead = cpool.tile([128, NC_ * NH], bf16)      # [h, (j, o)]

            ht = bigpool.tile([128, NC_ * B], bf16)        # [h, j*128 + g]  4MB
            aggT = bigpool.tile([128, NC_ * GRP], bf16)    # [(q,f'), j*32+grp] 1MB

            # head psum allocated up-front: accumulates across all slabs
            hd_ps_ctx = tc.tile_pool(name="hd_ps", bufs=1, space="PSUM")
            hd_ps = hd_ps_ctx.__enter__()
            pshead = hd_ps.tile([128, NH], f32)

            with (
                tc.tile_pool(name="xe_sb", bufs=4) as xe_pool,
                tc.tile_pool(name="z_ps", bufs=3, space="PSUM") as z_ps,
                tc.tile_pool(name="zr_sb", bufs=3) as zr_pool,
                tc.tile_pool(name="h_ps", bufs=1, space="PSUM") as h_ps,
            ):
                NB = SLABC // BLK                          # 8 blocks per slab
                NAS = len(ASSIST)                          # assisted blocks/slab
                htv = ht[:].rearrange("p (j grp q) -> p j grp q", grp=GRP, q=4)

                def stage_d_q(sd, q, eng="scalar", borrow=False):
                    # ht slab sd, quadrant q: relu(W_self x + W_neigh agg + b)
                    # last slab borrows z_ps ring tiles (stage A done) so its
                    # q-iterations pipeline instead of serializing on one tile
                    if borrow:
                        hpt = z_ps.tile([128, BLK], f32, tag="zps")
                        hp = hpt[:, 0:JS * GRP]
                    else:
                        hpt = h_ps.tile([128, JS * GRP], f32, tag="hps")
                        hp = hpt[:]
                    nc.tensor.matmul(
                        out=hp,
                        lhsT=wself[:, q * H:(q + 1) * H],
                        rhs=xt[:, sd * JS * GRP:(sd + 1) * JS * GRP],
                        start=True, stop=False)
                    nc.tensor.matmul(
                        out=hp,
                        lhsT=wneigh[:, q * H:(q + 1) * H],
                        rhs=aggT[:, sd * JS * GRP:(sd + 1) * JS * GRP],
                        start=False, stop=True)
                    ov = htv[:, sd * JS:(sd + 1) * JS, :, q]
                    iv = hp.rearrange("p (j grp) -> p j grp", grp=GRP)
                    if eng == "scalar":
                        nc.scalar.activation(out=ov, in_=iv, func=Relu,
                                             bias=bsage[:])
                    else:   # DVE; gpsimd cannot touch PSUM on real HW
                        nc.vector.tensor_scalar(
                            out=ov, in0=iv, scalar1=bsage[:], scalar2=0.0,
                            op0=Alu.add, op1=Alu.max)

                def stage_e(se):
                    # head accumulation over slab se's nodes
                    for jj in range(JS):
                        j = se * JS + jj
                        nc.tensor.matmul(
                            out=pshead[:],
                            lhsT=ht[:, j * B:(j + 1) * B],
                            rhs=whead[:, j * NH:(j + 1) * NH],
                            start=(j == 0), stop=(j == NC_ - 1),
                        )

                for s in range(NSLAB):
                    # ---- stage A+B: aggT slab = relu(max_d(W_pool@x[src]) + b) ----
                    # xe slab cols: (jj 16, grp 32, d 16); aggT cols: j*32+grp
                    xe = xe_pool.tile([128, SLABC], bf16, tag="xe")
                    nch = 4 if s == 0 else 2
                    for h2 in range(nch):  # chunks so compute starts earlier
                        cw = SLABC // nch
                        nc.sync.dma_start(
                            out=xe[:, h2 * cw:(h2 + 1) * cw],
                            in_=xe_d[:, s * SLABC + h2 * cw:
                                     s * SLABC + (h2 + 1) * cw])
                    if s == 0:
                        # deferred consts on the scalar-triggered DMA ring:
                        # in flight alongside xe slab 0, ready for stage D
                        nc.scalar.dma_start(out=wself[:], in_=wself_d[:])
                        nc.scalar.dma_start(out=wneigh[:], in_=wneigh_d[:])
                        nc.scalar.dma_start(out=bsage[:], in_=bsage_d[:])
                    # xt slab chunk (needed by stage D of this slab)
                    nc.scalar.dma_start(
                        out=xt[:, s * JS * GRP:(s + 1) * JS * GRP],
                        in_=xt_d[:, s * JS * GRP:(s + 1) * JS * GRP])
                    nc.sync.dma_start(
                        out=whead[:, s * JS * NH:(s + 1) * JS * NH],
                        in_=whead_d[:, s * JS * NH:(s + 1) * JS * NH])
                    zr = zr_pool.tile([128, NAS * BLK], bf16, tag="zr")
                    a0 = s * JS * GRP                      # aggT col offset
                    for blk in range(NB):   # 8 blocks of 1024 (64 nd, 16 d)
                        ps = z_ps.tile([128, BLK], f32, tag="zps")
                        for h2 in range(2):   # one matmul per PSUM bank
                            nc.tensor.matmul(
                                out=ps[:, h2 * 512:(h2 + 1) * 512],
                                lhsT=wpool[:],
                                rhs=xe[:, blk * BLK + h2 * 512:
                                        blk * BLK + (h2 + 1) * 512],
                                start=True, stop=True,
                            )
                        if blk in ASSIST:
                            # fused relu+bias drain on ACT; max-tree later
                            slot = ASSIST.index(blk)
                            nc.scalar.activation(
                                out=zr[:, slot * BLK:(slot + 1) * BLK],
                                in_=ps[:], func=Relu, bias=bpool[:])
                        else:
                            nc.vector.reduce_max(
                                out=aggT[:, a0 + blk * (BLK // DEG):
                                         a0 + (blk + 1) * (BLK // DEG)],
                                in_=ps[:].rearrange("p (n d) -> p n d", d=DEG),
                                axis=mybir.AxisListType.X)

                    # batched TT-max tree over d for assisted blocks (2x_1p)
                    ND = NAS * BLK // DEG                  # tree output cols
                    zrv = zr[:].rearrange("p (n d) -> p n d", d=DEG)
                    t1 = zr_pool.tile([128, ND * 8], bf16, tag="t1")
                    t1v = t1[:].rearrange("p (n d) -> p n d", d=8)
                    nc.vector.tensor_tensor(
                        out=t1v[:], in0=zrv[:, :, 0:8], in1=zrv[:, :, 8:16],
                        op=Alu.max)
                    t2 = zr_pool.tile([128, ND * 4], bf16, tag="t2")
                    t2v = t2[:].rearrange("p (n d) -> p n d", d=4)
                    nc.vector.tensor_tensor(
                        out=t2v[:], in0=t1v[:, :, 0:4], in1=t1v[:, :, 4:8],
                        op=Alu.max)
                    t3 = zr_pool.tile([128, ND * 2], bf16, tag="t3")
                    t3v = t3[:].rearrange("p (n d) -> p n d", d=2)
                    nc.vector.tensor_tensor(
                        out=t3v[:], in0=t2v[:, :, 0:2], in1=t2v[:, :, 2:4],
                        op=Alu.max)
                    # assisted aggT strips {1,3,5}: strided [128, 3, 64]
                    av = aggT[:].rearrange("p (b n) -> p b n", n=BLK // DEG)
                    t4a = t3v[:, :, 0].rearrange("p (b n) -> p b n", b=NAS)
                    t4b = t3v[:, :, 1].rearrange("p (b n) -> p b n", b=NAS)
                    nc.vector.tensor_tensor(
                        out=av[:, NB * s + 1:NB * s + 2 * NAS:2, :],
                        in0=t4a[:], in1=t4b[:], op=Alu.max)
                    # relu+bias for direct strips: {0,2,4} strided + {6,7};
                    # on GPSIMD (idle; SBUF-only access is legal there)
                    nc.gpsimd.tensor_scalar(
                        out=av[:, NB * s:NB * s + 2 * NAS:2, :],
                        in0=av[:, NB * s:NB * s + 2 * NAS:2, :],
                        scalar1=bpool[:], scalar2=0.0,
                        op0=Alu.add, op1=Alu.max)
                    nc.gpsimd.tensor_scalar(
                        out=av[:, NB * s + 2 * NAS:NB * (s + 1), :],
                        in0=av[:, NB * s + 2 * NAS:NB * (s + 1), :],
                        scalar1=bpool[:], scalar2=0.0,
                        op0=Alu.add, op1=Alu.max)

                    # ---- stage D + E for this slab ----
                    # last slab: alternate drains over ACT/DVE (bare tail)
                    last = s == NSLAB - 1
                    engs = (("scalar", "vector", "scalar", "vector")
                            if last else ("scalar",) * 4)
                    for q in range(4):
                        stage_d_q(s, q, eng=engs[q], borrow=last)
                    stage_e(s)

            if _dbg:
                nc.sync.dma_start(out=dbg_aggT_d[:], in_=aggT[:])
                nc.sync.dma_start(out=dbg_ht_d[:], in_=ht[:])

            # ---- output per-graph head partials; tail is summed on host ----
            with tc.tile_pool(name="tail", bufs=1) as tp:
                psf = tp.tile([128, NH], f32)
                nc.scalar.copy(out=psf[:], in_=pshead[:])
                nc.sync.dma_start(out=hpart_d[:], in_=psf[:])
            hd_ps_ctx.__exit__(None, None, None)
    nc.compile()
    return nc


def _make_in_maps(inputs):
    import ml_dtypes
    bf = ml_dtypes.bfloat16

    x = np.asarray(inputs["x"], np.float32)
    src = np.asarray(inputs["src"], np.int64)
    W_pool = np.asarray(inputs["W_pool"], np.float32)
    b_pool = np.asarray(inputs["b_pool"], np.float32)
    W_self = np.asarray(inputs["W_self"], np.float32)
    W_neigh = np.asarray(inputs["W_neigh"], np.float32)
    b_sage = np.asarray(inputs["b_sage"], np.float32)
    W_adv = np.asarray(inputs["W_adv"], np.float32)
    W_v1 = np.asarray(inputs["W_v1"], np.float32)

    # shared (replicated) tensors
    wpool_bd = np.kron(np.eye(4, dtype=np.float32), W_pool.T).astype(bf)  # [128,128]
    bpool = np.ascontiguousarray(np.tile(b_pool, 4)[:, None], np.float32)
    wself_bd = np.zeros((128, 4 * H), np.float32)
    wneigh_bd = np.zeros((128, 4 * H), np.float32)
    for q in range(4):
        wself_bd[q * 32:(q + 1) * 32, q * H:(q + 1) * H] = W_self.T
        wneigh_bd[q * 32:(q + 1) * 32, q * H:(q + 1) * H] = W_neigh.T
    bsage = np.ascontiguousarray(b_sage[:, None])
    W_cat = np.concatenate([W_adv, W_v1], axis=0)          # [76, 131072]
    W_cat = W_cat.reshape(NH, N, H)                        # [o, n, h]

    shared = {
        "wpool_bd": wpool_bd, "bpool": bpool,
        "wself_bd": wself_bd.astype(bf), "wneigh_bd": wneigh_bd.astype(bf),
        "bsage": bsage,
    }

    xbf = x.astype(bf)                                     # [128, 1024, 32]
    sidx = (src.reshape(B, N, DEG)
            - (np.arange(B, dtype=np.int64) * N)[:, None, None])  # local [0,N)
    garange = np.arange(B)[:, None, None]

    in_maps = []
    for c in range(NCORES):
        jsl = slice(NC_ * c, NC_ * (c + 1))
        xs = xbf[:, jsl, :]                                # [128g, 128j, 32f]
        # xt[(q,f), j*32+grp] = x[grp*4+q, 128c+j, f]
        xt = np.ascontiguousarray(
            xs.reshape(GRP, 4, NC_, F).transpose(1, 3, 2, 0)
            .reshape(128, NC_ * GRP))
        # xe[(q,f), ((j*32)+grp)*16+d] = x[g, src_local[g, 128c+j, d], f]
        sl = sidx[:, jsl, :]                               # [128g, 128j, 16d]
        xg = xbf[garange, sl, :]                           # [g, j, d, f]
        xe = np.ascontiguousarray(
            xg.reshape(GRP, 4, NC_, DEG, F).transpose(1, 4, 2, 0, 3)
            .reshape(128, NC_ * GRP * DEG))
        # whead[h, j*76+o] = W_cat[o, 128c+j, h]
        whead = np.ascontiguousarray(
            W_cat[:, jsl, :].transpose(2, 1, 0).reshape(H, NC_ * NH)).astype(bf)
        in_maps.append({"xt": xt, "xe": xe, "whead": whead, **shared})
    return in_maps


def _host_tail(hsum, inputs):
    """Dueling tail on the summed head partials [128, 76] (fp32, tiny)."""
    b_adv = np.asarray(inputs["b_adv"], np.float32)
    b_v1 = np.asarray(inputs["b_v1"], np.float32)
    W_v2 = np.asarray(inputs["W_v2"], np.float32)
    b_v2 = np.asarray(inputs["b_v2"], np.float32)
    W_v3 = np.asarray(inputs["W_v3"], np.float32)
    b_v3 = np.asarray(inputs["b_v3"], np.float32)

    adv = np.maximum(hsum[:, :NA] + b_adv, 0.0).reshape(B, 3, 4)
    val = np.maximum(hsum[:, NA:] + b_v1, 0.0)
    val = np.maximum(val @ W_v2.T + b_v2, 0.0)
    val = val @ W_v3.T + b_v3                               # [B, 1]
    return val[..., None] + adv - adv.mean(-1, keepdims=True)


def kernel(**inputs) -> np.ndarray:
    global LAST_RESULTS
    from concourse.bass_utils import run_bass_kernel_spmd

    if "nc" not in _CACHE:
        _CACHE["nc"] = _build_program()
    nc = _CACHE["nc"]
    in_maps = _make_in_maps(inputs)
    rr = run_bass_kernel_spmd(nc, in_maps, list(range(NCORES)))
    LAST_RESULTS = rr
    hsum = np.zeros((B, NH), np.float32)
    for c in range(NCORES):
        hsum += rr.results[c]["hpart"]
    return _host_tail(hsum, inputs).astype(np.float32)
